# revision 2
# baseline (speedup 1.0000x reference)
"""Trainium2 Bass kernel for nn_BoxCrossCategoryLoss (B = 4,194,304 rows).

Math: per row, each rel-id pair maps to a class code cls in [0,4)
((1,0)->0, (0,1)->1, (1,1)->2, (0,0)->3), and c = cls + 4*flag in [0,8).
The loss is a sum of per-recipe masked reductions over the joint key
K = cx + 8*cy + 64*cz in [0,512):

  positive recipes: loss -= sum_rows [K == key_r] * (v1[:,a]+v2[:,b]-v3[:,c])
  negative recipes: pick the (f+1)-th matching row per recipe (only when the
  recipe's mask has count > 0).

Distribution (data-parallel, 8 cores): rows are split into 8 contiguous
shards. Each core streams its shard (volumes + rel ids + flag, ~27 MiB) and
produces per-partition partial masked sums for the positive key-groups plus
per-recipe match counts for the negative recipes. The host reduces those
partials; negative recipes with count > 0 (O(1) rows) are resolved on the
host with the exact reference semantics. Counts are computed on-device for
every negative recipe, so the host fallback runs only when a mask is
non-empty.

Engine split per tile (cost-model tuned): the two HWDGE queues (SP + ACT)
share the streaming DMA; ACT converts rel ids to f16 via fused affine
copies using the factorization s*cls = (r0-0.5)*(4s*r1-3s) + 1.5s; POOL
assembles K and the mask sums in f16; DVE runs all 36 key compares as f16
tensor_scalar ops (4x mode), fusing the per-partition count reduction for
the negative recipes.
"""
import numpy as np

import concourse.bass as bass
import concourse.mybir as mybir
import concourse.tile as tile
from concourse.bass_utils import run_bass_kernel_spmd

F32 = mybir.dt.float32
F16 = mybir.dt.float16
I32 = mybir.dt.int32
ALU = mybir.AluOpType
AF = mybir.ActivationFunctionType

N_CORES = 8
B = 4_194_304
P = 128
ROWS_PER_CORE = B // N_CORES          # 524288
R = ROWS_PER_CORE // P                # 4096 rows per partition
N_TILE = 512                          # rows per partition per tile
T = R // N_TILE                       # 8 tiles
ACT_LOADS = ("v2t", "xzt")            # tensors loaded via the ACT HWDGE queue
DVE_ADDS = 2                          # mask-sum adds placed on DVE (rest POOL)

LOSS_RECIPE = [(0, 4, 4), (0, 6, 4), (1, 5, 5), (1, 6, 5), (2, 4, 4), (2, 5, 5),
               (2, 6, 6), (2, 7, 7), (4, 0, 4), (4, 2, 4), (5, 1, 5), (5, 2, 5),
               (6, 2, 6), (7, 2, 7)]
NEG_LOSS_RECIPE = [(0, 4, 1), (0, 4, 2), (0, 6, 1), (0, 6, 2), (1, 5, 0), (1, 5, 2),
                   (1, 6, 0), (1, 6, 2), (2, 4, 1), (2, 4, 2), (2, 5, 0), (2, 5, 2),
                   (4, 0, 1), (4, 0, 2), (4, 2, 1), (4, 2, 2), (5, 1, 0), (5, 1, 2),
                   (5, 2, 0), (5, 2, 2), (2, 7, 2), (7, 2, 2)]

LOG_HALF = -0.6931471805599453


def _key(xy, yz, xz):
    return xy + 8 * yz + 64 * xz


def _pos_sets():
    """Positive recipes grouped by (xy//4, yz//4, xz//4): each group shares
    the term v1[:,a] + v2[:,b] - v3[:,c]."""
    groups = {}
    for xy, yz, xz in LOSS_RECIPE:
        groups.setdefault((xy // 4, yz // 4, xz // 4), []).append(_key(xy, yz, xz))
    return [(ks, abc) for abc, ks in sorted(groups.items())]


POS_SETS = _pos_sets()
NEG_KEYS = [_key(*r) for r in NEG_LOSS_RECIPE]
N_SETS = len(POS_SETS)
N_NEG = len(NEG_KEYS)


# --------------------------------------------------------------------------
# Workaround for the toolchain's 1-sync-wait-per-instruction codegen limit:
# spread multi-wait instructions' semaphore waits across same-engine NOPs
# emitted immediately before them (same-queue order preserves semantics).
def _split_multi_waits(nc):
    def builder(engine):
        e = mybir.EngineType
        return {e.SP: nc.sync, e.DVE: nc.vector, e.Activation: nc.scalar,
                e.PE: nc.tensor, e.Pool: nc.gpsimd}[engine]

    f = nc.m.functions[0]
    tail = nc.cur_bb.bb

    def process(b):
        snapshot = list(b.instructions)
        changed = False
        new_list = []
        for ins in snapshot:
            si = ins.sync_info
            if si is not None and len(si.on_wait) > 1:
                waits = list(si.on_wait)
                for w in waits[:-1]:
                    nop = builder(ins.engine).nop(nofuse=True, hint="waitsplit").ins
                    tl = list(tail.instructions)
                    assert tl and tl[-1].name == nop.name
                    tail.instructions = tl[:-1]
                    nop.sync_info = mybir.SyncInfo(on_wait=[w], on_update=[])
                    new_list.append(nop)
                ins.sync_info = mybir.SyncInfo(
                    on_wait=[waits[-1]], on_update=list(si.on_update or []))
                changed = True
            new_list.append(ins)
        if changed:
            b.instructions = new_list
        for sub in getattr(b, "blocks", []) or []:
            process(sub)

    for b in f.blocks:
        process(b)


def _build_nc():
    rows = P * R
    nc = bass.Bass()
    v1 = nc.declare_dram_parameter("volume1", [rows, 2], F32, isOutput=False)
    v2 = nc.declare_dram_parameter("volume2", [rows, 2], F32, isOutput=False)
    v3 = nc.declare_dram_parameter("volume3", [rows, 2], F32, isOutput=False)
    xy = nc.declare_dram_parameter("xy_rel_id", [rows, 2], I32, isOutput=False)
    yz = nc.declare_dram_parameter("yz_rel_id", [rows, 2], I32, isOutput=False)
    xz = nc.declare_dram_parameter("xz_rel_id", [rows, 2], I32, isOutput=False)
    fl = nc.declare_dram_parameter("flag", [rows], I32, isOutput=False)
    pos_out = nc.declare_dram_parameter("pos", [P, T * N_SETS], F32, isOutput=True)
    cnt_out = nc.declare_dram_parameter("cnt", [P, T * N_NEG], F32, isOutput=True)

    v1r = v1.rearrange("(p n) m -> p n m", p=P)
    v2r = v2.rearrange("(p n) m -> p n m", p=P)
    v3r = v3.rearrange("(p n) m -> p n m", p=P)
    xyr = xy.rearrange("(p n) m -> p n m", p=P)
    yzr = yz.rearrange("(p n) m -> p n m", p=P)
    xzr = xz.rearrange("(p n) m -> p n m", p=P)
    flr = fl.rearrange("(p n) -> p n", p=P)
    N = N_TILE

    with tile.TileContext(nc) as tc:
        with tc.tile_pool(name="io", bufs=3) as io, \
             tc.tile_pool(name="scr", bufs=2) as scr, \
             tc.tile_pool(name="accs", bufs=1) as accs:
            pos_acc = accs.tile([P, T * N_SETS], F32)
            cnt_acc = accs.tile([P, T * N_NEG], F32)

            for j in range(T):
                sl = bass.ts(j, N)
                v1t = io.tile([P, N, 2], F32, tag="v1t")
                v2t = io.tile([P, N, 2], F32, tag="v2t")
                v3t = io.tile([P, N, 2], F32, tag="v3t")
                xyt = io.tile([P, N, 2], I32, tag="xyt")
                yzt = io.tile([P, N, 2], I32, tag="yzt")
                xzt = io.tile([P, N, 2], I32, tag="xzt")
                flt = io.tile([P, N], I32, tag="flt")
                for nm, dst, src_ap in (("v1t", v1t, v1r[:, sl, :]),
                                        ("v2t", v2t, v2r[:, sl, :]),
                                        ("v3t", v3t, v3r[:, sl, :]),
                                        ("xyt", xyt, xyr[:, sl, :]),
                                        ("yzt", yzt, yzr[:, sl, :]),
                                        ("xzt", xzt, xzr[:, sl, :]),
                                        ("flt", flt, flr[:, sl])):
                    eng = nc.scalar if nm in ACT_LOADS else nc.sync
                    eng.dma_start(dst[:], src_ap)

                # K = wx + wy + wz + (292*flag + 109.5), w = (r0-.5)(4s*r1-3s)
                us, vs = [], []
                for nm, rel, s in (("x", xyt, 1.0), ("y", yzt, 8.0), ("z", xzt, 64.0)):
                    u = scr.tile([P, N], F16, tag=f"u{nm}")
                    v = scr.tile([P, N], F16, tag=f"v{nm}")
                    nc.scalar.activation(u[:], rel[:, :, 0], AF.Copy, bias=-0.5, scale=1.0)
                    nc.scalar.activation(v[:], rel[:, :, 1], AF.Copy, bias=-3.0 * s, scale=4.0 * s)
                    us.append(u); vs.append(v)
                ff = scr.tile([P, N], F16, tag="ff")
                nc.scalar.activation(ff[:], flt[:], AF.Copy, bias=109.5, scale=292.0)
                for u, v in zip(us, vs):
                    nc.gpsimd.tensor_tensor(u[:], u[:], v[:], ALU.mult)
                nc.gpsimd.tensor_tensor(us[0][:], us[0][:], us[1][:], ALU.add)
                nc.gpsimd.tensor_tensor(us[2][:], us[2][:], ff[:], ALU.add)
                K = scr.tile([P, N], F16, tag="K")
                nc.gpsimd.tensor_tensor(K[:], us[0][:], us[2][:], ALU.add)

                # positive branch: masked sums per key-group
                adds_left = DVE_ADDS
                for s, (keys, (a, b, c)) in enumerate(POS_SETS):
                    M = scr.tile([P, N], F16, tag=f"M{s}")
                    nc.vector.tensor_scalar(M[:], K[:], float(keys[0]), None, ALU.is_equal)
                    for ki, k in enumerate(keys[1:]):
                        CMP = scr.tile([P, N], F16, tag=f"CMP{ki % 4}")
                        nc.vector.tensor_scalar(CMP[:], K[:], float(k), None, ALU.is_equal)
                        if adds_left > 0:
                            nc.vector.tensor_tensor(M[:], M[:], CMP[:], ALU.add)
                            adds_left -= 1
                        else:
                            nc.gpsimd.tensor_tensor(M[:], M[:], CMP[:], ALU.add)
                    TT = scr.tile([P, N], F32, tag=f"T{s}")
                    nc.gpsimd.tensor_tensor(TT[:], v1t[:, :, a], v2t[:, :, b], ALU.add)
                    nc.gpsimd.tensor_tensor(TT[:], TT[:], v3t[:, :, c], ALU.subtract)
                    D = scr.tile([P, N], F32, tag="D")
                    # scalar_tensor_tensor is DVE-only in this codegen
                    nc.vector.scalar_tensor_tensor(
                        D[:], TT[:], 1.0, M[:], ALU.mult, ALU.mult,
                        accum_out=pos_acc[:, j * N_SETS + s:j * N_SETS + s + 1])

                # negative branch: per-recipe match counts (compare + fused
                # per-partition sum; op1 is the reduction operator)
                NS = scr.tile([P, N], F16, tag="NS")
                for r, k in enumerate(NEG_KEYS):
                    nc.vector.tensor_scalar(
                        NS[:], K[:], float(k), None, ALU.is_equal, ALU.add,
                        accum_out=cnt_acc[:, j * N_NEG + r:j * N_NEG + r + 1])

            nc.sync.dma_start(pos_out[:], pos_acc[:])
            nc.scalar.dma_start(cnt_out[:], cnt_acc[:])

    _split_multi_waits(nc)
    return nc


_NC_CACHE = None


def _get_nc():
    global _NC_CACHE
    if _NC_CACHE is None:
        _NC_CACHE = _build_nc()
    return _NC_CACHE


# ------------------------- host-side helpers ------------------------------
def _codes_np(rel, flag):
    r0, r1 = rel[:, 0], rel[:, 1]
    cls = np.where((r0 == 1) & (r1 == 0), 0,
          np.where((r0 == 0) & (r1 == 1), 1,
          np.where((r0 == 1) & (r1 == 1), 2, 3)))
    return cls + 4 * flag


def _log1mexp_np(x):
    x = np.asarray(x, dtype=np.float32)
    return np.where(x > np.float32(LOG_HALF),
                    np.log(-np.expm1(x)), np.log1p(-np.exp(x))).astype(np.float32)


def _neg_term_host(volume1, volume2, volume3, cx, cy, cz, xy, yz, xz):
    """Exact reference semantics for one negative recipe (used only when the
    device-computed count for that recipe is non-zero)."""
    m = (cx == xy) & (cy == yz) & (cz == xz)
    cs = np.cumsum(m.astype(np.int32))
    count = int(cs[-1])
    if count <= 0:
        return np.float32(0.0)
    f1, f2, f3 = xy // 4, yz // 4, xz // 4
    i1 = int(np.argmax(cs == f1 + 1))
    i2 = int(np.argmax(cs == f2 + 1))
    i3 = int(np.argmax(cs == f3 + 1))
    term = (volume1[i1].astype(np.float32)
            + volume2[i2].astype(np.float32)
            - _log1mexp_np(volume3[i3])).sum(dtype=np.float32)
    return np.float32(term)


def kernel(volume1, volume2, volume3, xy_rel_id, yz_rel_id, xz_rel_id, flag):
    v1 = np.ascontiguousarray(np.asarray(volume1, dtype=np.float32))
    v2 = np.ascontiguousarray(np.asarray(volume2, dtype=np.float32))
    v3 = np.ascontiguousarray(np.asarray(volume3, dtype=np.float32))
    xy = np.ascontiguousarray(np.asarray(xy_rel_id).astype(np.int32, copy=False))
    yz = np.ascontiguousarray(np.asarray(yz_rel_id).astype(np.int32, copy=False))
    xz = np.ascontiguousarray(np.asarray(xz_rel_id).astype(np.int32, copy=False))
    fl = np.ascontiguousarray(np.asarray(flag).astype(np.int32, copy=False))
    assert v1.shape == (B, 2) and fl.shape == (B,)

    nc = _get_nc()
    S = ROWS_PER_CORE
    in_maps = [{
        "volume1": v1[c * S:(c + 1) * S],
        "volume2": v2[c * S:(c + 1) * S],
        "volume3": v3[c * S:(c + 1) * S],
        "xy_rel_id": xy[c * S:(c + 1) * S],
        "yz_rel_id": yz[c * S:(c + 1) * S],
        "xz_rel_id": xz[c * S:(c + 1) * S],
        "flag": fl[c * S:(c + 1) * S],
    } for c in range(N_CORES)]

    res = run_bass_kernel_spmd(nc, in_maps, core_ids=list(range(N_CORES)))

    pos_total = np.float32(0.0)
    counts = np.zeros(N_NEG, dtype=np.float64)
    for c in range(N_CORES):
        pos = res.results[c]["pos"]          # [P, T * N_SETS]
        cnt = res.results[c]["cnt"]          # [P, T * N_NEG]
        pos_total = np.float32(pos_total + pos.sum(dtype=np.float64))
        counts += cnt.reshape(P, T, N_NEG).sum(axis=(0, 1))

    loss = np.float32(0.0) - pos_total

    if np.any(counts > 0):
        # some negative-recipe mask is non-empty: resolve those recipes on
        # the host with exact reference semantics
        cx = _codes_np(xy, fl)
        cy = _codes_np(yz, fl)
        cz = _codes_np(xz, fl)
        for r, (rxy, ryz, rxz) in enumerate(NEG_LOSS_RECIPE):
            if counts[r] > 0:
                loss = np.float32(loss - _neg_term_host(v1, v2, v3, cx, cy, cz,
                                                        rxy, ryz, rxz))

    return np.float32(loss)


# revision 3
# speedup vs baseline: 1.0156x; 1.0156x over previous
"""Trainium2 Bass kernel for nn_BoxCrossCategoryLoss (B = 4,194,304 rows).

Math: per row, each rel-id pair maps to a class code cls in [0,4)
((1,0)->0, (0,1)->1, (1,1)->2, (0,0)->3), and c = cls + 4*flag in [0,8).
The loss is a sum of per-recipe masked reductions over the joint key
K = cx + 8*cy + 64*cz in [0,512):

  positive recipes: loss -= sum_rows [K == key_r] * (v1[:,a]+v2[:,b]-v3[:,c])
  negative recipes: pick the (f+1)-th matching row per recipe (only when the
  recipe's mask has count > 0).

Distribution (data-parallel, 8 cores): rows are split into 8 contiguous
shards. Each core streams its shard (volumes + rel ids + flag, ~27 MiB) and
produces per-partition partial masked sums for the positive key-groups plus
per-recipe match counts for the negative recipes. The host reduces those
partials; negative recipes with count > 0 (O(1) rows) are resolved on the
host with the exact reference semantics. Counts are computed on-device for
every negative recipe, so the host fallback runs only when a mask is
non-empty.

Engine split per tile (cost-model tuned): the two HWDGE queues (SP + ACT)
share the streaming DMA; ACT converts rel ids to f16 via fused affine
copies using the factorization s*cls = (r0-0.5)*(4s*r1-3s) + 1.5s; POOL
assembles K and the mask sums in f16; DVE runs all 36 key compares as f16
tensor_scalar ops (4x mode), fusing the per-partition count reduction for
the negative recipes.
"""
import numpy as np

import concourse.bass as bass
import concourse.mybir as mybir
import concourse.tile as tile
from concourse.bass_utils import run_bass_kernel_spmd

F32 = mybir.dt.float32
F16 = mybir.dt.float16
I32 = mybir.dt.int32
ALU = mybir.AluOpType
AF = mybir.ActivationFunctionType

N_CORES = 8
B = 4_194_304
P = 128
ROWS_PER_CORE = B // N_CORES          # 524288
R = ROWS_PER_CORE // P                # 4096 rows per partition
N_TILE = 512                          # rows per partition per tile
T = R // N_TILE                       # 8 tiles
ACT_LOADS = ("xyt", "xzt")            # tensors loaded via the ACT HWDGE queue
DVE_ADDS = 2                          # mask-sum adds placed on DVE (rest POOL)

LOSS_RECIPE = [(0, 4, 4), (0, 6, 4), (1, 5, 5), (1, 6, 5), (2, 4, 4), (2, 5, 5),
               (2, 6, 6), (2, 7, 7), (4, 0, 4), (4, 2, 4), (5, 1, 5), (5, 2, 5),
               (6, 2, 6), (7, 2, 7)]
NEG_LOSS_RECIPE = [(0, 4, 1), (0, 4, 2), (0, 6, 1), (0, 6, 2), (1, 5, 0), (1, 5, 2),
                   (1, 6, 0), (1, 6, 2), (2, 4, 1), (2, 4, 2), (2, 5, 0), (2, 5, 2),
                   (4, 0, 1), (4, 0, 2), (4, 2, 1), (4, 2, 2), (5, 1, 0), (5, 1, 2),
                   (5, 2, 0), (5, 2, 2), (2, 7, 2), (7, 2, 2)]

LOG_HALF = -0.6931471805599453


def _key(xy, yz, xz):
    return xy + 8 * yz + 64 * xz


def _pos_sets():
    """Positive recipes grouped by (xy//4, yz//4, xz//4): each group shares
    the term v1[:,a] + v2[:,b] - v3[:,c]."""
    groups = {}
    for xy, yz, xz in LOSS_RECIPE:
        groups.setdefault((xy // 4, yz // 4, xz // 4), []).append(_key(xy, yz, xz))
    return [(ks, abc) for abc, ks in sorted(groups.items())]


POS_SETS = _pos_sets()
NEG_KEYS = [_key(*r) for r in NEG_LOSS_RECIPE]
N_SETS = len(POS_SETS)
N_NEG = len(NEG_KEYS)


# --------------------------------------------------------------------------
# Workaround for the toolchain's 1-sync-wait-per-instruction codegen limit:
# spread multi-wait instructions' semaphore waits across same-engine NOPs
# emitted immediately before them (same-queue order preserves semantics).
def _split_multi_waits(nc):
    def builder(engine):
        e = mybir.EngineType
        return {e.SP: nc.sync, e.DVE: nc.vector, e.Activation: nc.scalar,
                e.PE: nc.tensor, e.Pool: nc.gpsimd}[engine]

    f = nc.m.functions[0]
    tail = nc.cur_bb.bb

    def process(b):
        snapshot = list(b.instructions)
        changed = False
        new_list = []
        for ins in snapshot:
            si = ins.sync_info
            if si is not None and len(si.on_wait) > 1:
                waits = list(si.on_wait)
                for w in waits[:-1]:
                    nop = builder(ins.engine).nop(nofuse=True, hint="waitsplit").ins
                    tl = list(tail.instructions)
                    assert tl and tl[-1].name == nop.name
                    tail.instructions = tl[:-1]
                    nop.sync_info = mybir.SyncInfo(on_wait=[w], on_update=[])
                    new_list.append(nop)
                ins.sync_info = mybir.SyncInfo(
                    on_wait=[waits[-1]], on_update=list(si.on_update or []))
                changed = True
            new_list.append(ins)
        if changed:
            b.instructions = new_list
        for sub in getattr(b, "blocks", []) or []:
            process(sub)

    for b in f.blocks:
        process(b)


def _build_nc():
    rows = P * R
    nc = bass.Bass()
    v1 = nc.declare_dram_parameter("volume1", [rows, 2], F32, isOutput=False)
    v2 = nc.declare_dram_parameter("volume2", [rows, 2], F32, isOutput=False)
    v3 = nc.declare_dram_parameter("volume3", [rows, 2], F32, isOutput=False)
    xy = nc.declare_dram_parameter("xy_rel_id", [rows, 2], I32, isOutput=False)
    yz = nc.declare_dram_parameter("yz_rel_id", [rows, 2], I32, isOutput=False)
    xz = nc.declare_dram_parameter("xz_rel_id", [rows, 2], I32, isOutput=False)
    fl = nc.declare_dram_parameter("flag", [rows], I32, isOutput=False)
    pos_out = nc.declare_dram_parameter("pos", [P, T * N_SETS], F32, isOutput=True)
    cnt_out = nc.declare_dram_parameter("cnt", [P, T * N_NEG], F32, isOutput=True)

    v1r = v1.rearrange("(p n) m -> p n m", p=P)
    v2r = v2.rearrange("(p n) m -> p n m", p=P)
    v3r = v3.rearrange("(p n) m -> p n m", p=P)
    xyr = xy.rearrange("(p n) m -> p n m", p=P)
    yzr = yz.rearrange("(p n) m -> p n m", p=P)
    xzr = xz.rearrange("(p n) m -> p n m", p=P)
    flr = fl.rearrange("(p n) -> p n", p=P)
    N = N_TILE

    with tile.TileContext(nc) as tc:
        with tc.tile_pool(name="io", bufs=3) as io, \
             tc.tile_pool(name="scr", bufs=2) as scr, \
             tc.tile_pool(name="accs", bufs=1) as accs:
            pos_acc = accs.tile([P, T * N_SETS], F32)
            cnt_acc = accs.tile([P, T * N_NEG], F32)

            for j in range(T):
                sl = bass.ts(j, N)
                v1t = io.tile([P, N, 2], F32, tag="v1t")
                v2t = io.tile([P, N, 2], F32, tag="v2t")
                v3t = io.tile([P, N, 2], F32, tag="v3t")
                xyt = io.tile([P, N, 2], I32, tag="xyt")
                yzt = io.tile([P, N, 2], I32, tag="yzt")
                xzt = io.tile([P, N, 2], I32, tag="xzt")
                flt = io.tile([P, N], I32, tag="flt")
                for nm, dst, src_ap in (("v1t", v1t, v1r[:, sl, :]),
                                        ("v2t", v2t, v2r[:, sl, :]),
                                        ("v3t", v3t, v3r[:, sl, :]),
                                        ("xyt", xyt, xyr[:, sl, :]),
                                        ("yzt", yzt, yzr[:, sl, :]),
                                        ("xzt", xzt, xzr[:, sl, :]),
                                        ("flt", flt, flr[:, sl])):
                    eng = nc.scalar if nm in ACT_LOADS else nc.sync
                    eng.dma_start(dst[:], src_ap)

                # K = wx + wy + wz + (292*flag + 109.5), w = (r0-.5)(4s*r1-3s)
                us, vs = [], []
                for nm, rel, s in (("x", xyt, 1.0), ("y", yzt, 8.0), ("z", xzt, 64.0)):
                    u = scr.tile([P, N], F16, tag=f"u{nm}")
                    v = scr.tile([P, N], F16, tag=f"v{nm}")
                    nc.scalar.activation(u[:], rel[:, :, 0], AF.Copy, bias=-0.5, scale=1.0)
                    nc.scalar.activation(v[:], rel[:, :, 1], AF.Copy, bias=-3.0 * s, scale=4.0 * s)
                    us.append(u); vs.append(v)
                ff = scr.tile([P, N], F16, tag="ff")
                nc.scalar.activation(ff[:], flt[:], AF.Copy, bias=109.5, scale=292.0)
                for u, v in zip(us, vs):
                    nc.gpsimd.tensor_tensor(u[:], u[:], v[:], ALU.mult)
                nc.gpsimd.tensor_tensor(us[0][:], us[0][:], us[1][:], ALU.add)
                nc.gpsimd.tensor_tensor(us[2][:], us[2][:], ff[:], ALU.add)
                K = scr.tile([P, N], F16, tag="K")
                nc.gpsimd.tensor_tensor(K[:], us[0][:], us[2][:], ALU.add)

                # positive branch: masked sums per key-group
                adds_left = DVE_ADDS
                for s, (keys, (a, b, c)) in enumerate(POS_SETS):
                    M = scr.tile([P, N], F16, tag=f"M{s}")
                    nc.vector.tensor_scalar(M[:], K[:], float(keys[0]), None, ALU.is_equal)
                    for ki, k in enumerate(keys[1:]):
                        CMP = scr.tile([P, N], F16, tag=f"CMP{ki % 4}")
                        nc.vector.tensor_scalar(CMP[:], K[:], float(k), None, ALU.is_equal)
                        if adds_left > 0:
                            nc.vector.tensor_tensor(M[:], M[:], CMP[:], ALU.add)
                            adds_left -= 1
                        else:
                            nc.gpsimd.tensor_tensor(M[:], M[:], CMP[:], ALU.add)
                    TT = scr.tile([P, N], F32, tag=f"T{s}")
                    nc.gpsimd.tensor_tensor(TT[:], v1t[:, :, a], v2t[:, :, b], ALU.add)
                    nc.gpsimd.tensor_tensor(TT[:], TT[:], v3t[:, :, c], ALU.subtract)
                    D = scr.tile([P, N], F32, tag="D")
                    # scalar_tensor_tensor is DVE-only in this codegen
                    nc.vector.scalar_tensor_tensor(
                        D[:], TT[:], 1.0, M[:], ALU.mult, ALU.mult,
                        accum_out=pos_acc[:, j * N_SETS + s:j * N_SETS + s + 1])

                # negative branch: per-recipe match counts (compare + fused
                # per-partition sum; op1 is the reduction operator)
                NS = scr.tile([P, N], F16, tag="NS")
                for r, k in enumerate(NEG_KEYS):
                    nc.vector.tensor_scalar(
                        NS[:], K[:], float(k), None, ALU.is_equal, ALU.add,
                        accum_out=cnt_acc[:, j * N_NEG + r:j * N_NEG + r + 1])

            nc.sync.dma_start(pos_out[:], pos_acc[:])
            nc.scalar.dma_start(cnt_out[:], cnt_acc[:])

    _split_multi_waits(nc)
    return nc


_NC_CACHE = None


def _get_nc():
    global _NC_CACHE
    if _NC_CACHE is None:
        _NC_CACHE = _build_nc()
    return _NC_CACHE


# ------------------------- host-side helpers ------------------------------
def _codes_np(rel, flag):
    r0, r1 = rel[:, 0], rel[:, 1]
    cls = np.where((r0 == 1) & (r1 == 0), 0,
          np.where((r0 == 0) & (r1 == 1), 1,
          np.where((r0 == 1) & (r1 == 1), 2, 3)))
    return cls + 4 * flag


def _log1mexp_np(x):
    x = np.asarray(x, dtype=np.float32)
    return np.where(x > np.float32(LOG_HALF),
                    np.log(-np.expm1(x)), np.log1p(-np.exp(x))).astype(np.float32)


def _neg_term_host(volume1, volume2, volume3, cx, cy, cz, xy, yz, xz):
    """Exact reference semantics for one negative recipe (used only when the
    device-computed count for that recipe is non-zero)."""
    m = (cx == xy) & (cy == yz) & (cz == xz)
    cs = np.cumsum(m.astype(np.int32))
    count = int(cs[-1])
    if count <= 0:
        return np.float32(0.0)
    f1, f2, f3 = xy // 4, yz // 4, xz // 4
    i1 = int(np.argmax(cs == f1 + 1))
    i2 = int(np.argmax(cs == f2 + 1))
    i3 = int(np.argmax(cs == f3 + 1))
    term = (volume1[i1].astype(np.float32)
            + volume2[i2].astype(np.float32)
            - _log1mexp_np(volume3[i3])).sum(dtype=np.float32)
    return np.float32(term)


def kernel(volume1, volume2, volume3, xy_rel_id, yz_rel_id, xz_rel_id, flag):
    v1 = np.ascontiguousarray(np.asarray(volume1, dtype=np.float32))
    v2 = np.ascontiguousarray(np.asarray(volume2, dtype=np.float32))
    v3 = np.ascontiguousarray(np.asarray(volume3, dtype=np.float32))
    xy = np.ascontiguousarray(np.asarray(xy_rel_id).astype(np.int32, copy=False))
    yz = np.ascontiguousarray(np.asarray(yz_rel_id).astype(np.int32, copy=False))
    xz = np.ascontiguousarray(np.asarray(xz_rel_id).astype(np.int32, copy=False))
    fl = np.ascontiguousarray(np.asarray(flag).astype(np.int32, copy=False))
    assert v1.shape == (B, 2) and fl.shape == (B,)

    nc = _get_nc()
    S = ROWS_PER_CORE
    in_maps = [{
        "volume1": v1[c * S:(c + 1) * S],
        "volume2": v2[c * S:(c + 1) * S],
        "volume3": v3[c * S:(c + 1) * S],
        "xy_rel_id": xy[c * S:(c + 1) * S],
        "yz_rel_id": yz[c * S:(c + 1) * S],
        "xz_rel_id": xz[c * S:(c + 1) * S],
        "flag": fl[c * S:(c + 1) * S],
    } for c in range(N_CORES)]

    res = run_bass_kernel_spmd(nc, in_maps, core_ids=list(range(N_CORES)))

    pos_total = np.float32(0.0)
    counts = np.zeros(N_NEG, dtype=np.float64)
    for c in range(N_CORES):
        pos = res.results[c]["pos"]          # [P, T * N_SETS]
        cnt = res.results[c]["cnt"]          # [P, T * N_NEG]
        pos_total = np.float32(pos_total + pos.sum(dtype=np.float64))
        counts += cnt.reshape(P, T, N_NEG).sum(axis=(0, 1))

    loss = np.float32(0.0) - pos_total

    if np.any(counts > 0):
        # some negative-recipe mask is non-empty: resolve those recipes on
        # the host with exact reference semantics
        cx = _codes_np(xy, fl)
        cy = _codes_np(yz, fl)
        cz = _codes_np(xz, fl)
        for r, (rxy, ryz, rxz) in enumerate(NEG_LOSS_RECIPE):
            if counts[r] > 0:
                loss = np.float32(loss - _neg_term_host(v1, v2, v3, cx, cy, cz,
                                                        rxy, ryz, rxz))

    return np.float32(loss)


# revision 13
# speedup vs baseline: 1.0403x; 1.0243x over previous
"""Trainium2 Bass kernel for nn_BoxCrossCategoryLoss (B = 4,194,304 rows).

Math: per row, each rel-id pair maps to a class code cls in [0,4)
((1,0)->0, (0,1)->1, (1,1)->2, (0,0)->3), and c = cls + 4*flag in [0,8).
The loss is a sum of per-recipe masked reductions over the joint key
K = cx + 8*cy + 64*cz in [0,512):

  positive recipes: loss -= sum_rows [K == key_r] * (v1[:,a]+v2[:,b]-v3[:,c])
  negative recipes: pick the (f+1)-th matching row per recipe (only when the
  recipe's mask has count > 0).

Distribution (data-parallel, 8 cores): rows are split into 8 contiguous
shards. Each core streams its shard (volumes + rel ids + flag, ~27 MiB) and
produces per-partition partial masked sums for the positive key-groups plus
per-recipe match counts for the negative recipes. The host reduces those
partials; negative recipes with count > 0 (O(1) rows) are resolved on the
host with the exact reference semantics. Counts are computed on-device for
every negative recipe, so the host fallback runs only when a mask is
non-empty.

Engine split per tile (cost-model tuned): the two HWDGE queues (SP + ACT)
share the streaming DMA; ACT converts rel ids to f16 via fused affine
copies using the factorization s*cls = (r0-0.5)*(4s*r1-3s) + 1.5s; POOL
assembles K and the mask sums in f16; DVE runs all 36 key compares as f16
tensor_scalar ops (4x mode), fusing the per-partition count reduction for
the negative recipes.
"""
import numpy as np

import concourse.bass as bass
import concourse.mybir as mybir
import concourse.tile as tile
from concourse.bass_utils import run_bass_kernel_spmd

F32 = mybir.dt.float32
F16 = mybir.dt.float16
I32 = mybir.dt.int32
ALU = mybir.AluOpType
AF = mybir.ActivationFunctionType

N_CORES = 8
B = 4_194_304
P = 128
ROWS_PER_CORE = B // N_CORES          # 524288
R = ROWS_PER_CORE // P                # 4096 rows per partition
N_TILE = 512                          # rows per partition per tile
T = R // N_TILE                       # 8 tiles
ACT_LOADS = ("xyt", "xzt")            # tensors loaded via the ACT HWDGE queue
DVE_ADDS = 2                          # mask-sum adds placed on DVE (rest POOL)
NEG_CHUNK = 1024                      # rows per neg-count compare (coarser
                                      # than N_TILE: amortizes DVE per-op cost)
PROLOGUE_SLICES = [(0, 512)]          # first-tile split (plain: splits hurt)
PROLOGUE_ROWS = 512

LOSS_RECIPE = [(0, 4, 4), (0, 6, 4), (1, 5, 5), (1, 6, 5), (2, 4, 4), (2, 5, 5),
               (2, 6, 6), (2, 7, 7), (4, 0, 4), (4, 2, 4), (5, 1, 5), (5, 2, 5),
               (6, 2, 6), (7, 2, 7)]
NEG_LOSS_RECIPE = [(0, 4, 1), (0, 4, 2), (0, 6, 1), (0, 6, 2), (1, 5, 0), (1, 5, 2),
                   (1, 6, 0), (1, 6, 2), (2, 4, 1), (2, 4, 2), (2, 5, 0), (2, 5, 2),
                   (4, 0, 1), (4, 0, 2), (4, 2, 1), (4, 2, 2), (5, 1, 0), (5, 1, 2),
                   (5, 2, 0), (5, 2, 2), (2, 7, 2), (7, 2, 2)]

LOG_HALF = -0.6931471805599453


def _key(xy, yz, xz):
    return xy + 8 * yz + 64 * xz


def _pos_sets():
    """Positive recipes grouped by (xy//4, yz//4, xz//4): each group shares
    the term v1[:,a] + v2[:,b] - v3[:,c]."""
    groups = {}
    for xy, yz, xz in LOSS_RECIPE:
        groups.setdefault((xy // 4, yz // 4, xz // 4), []).append(_key(xy, yz, xz))
    return [(ks, abc) for abc, ks in sorted(groups.items())]


POS_SETS = _pos_sets()
NEG_KEYS = [_key(*r) for r in NEG_LOSS_RECIPE]
N_SETS = len(POS_SETS)
N_NEG = len(NEG_KEYS)


# --------------------------------------------------------------------------
# Workaround for the toolchain's 1-sync-wait-per-instruction codegen limit:
# spread multi-wait instructions' semaphore waits across same-engine NOPs
# emitted immediately before them (same-queue order preserves semantics).
def _split_multi_waits(nc):
    def builder(engine):
        e = mybir.EngineType
        return {e.SP: nc.sync, e.DVE: nc.vector, e.Activation: nc.scalar,
                e.PE: nc.tensor, e.Pool: nc.gpsimd}[engine]

    f = nc.m.functions[0]
    tail = nc.cur_bb.bb

    def process(b):
        snapshot = list(b.instructions)
        changed = False
        new_list = []
        for ins in snapshot:
            si = ins.sync_info
            if si is not None and len(si.on_wait) > 1:
                waits = list(si.on_wait)
                for w in waits[:-1]:
                    nop = builder(ins.engine).nop(nofuse=True, hint="waitsplit").ins
                    tl = list(tail.instructions)
                    assert tl and tl[-1].name == nop.name
                    tail.instructions = tl[:-1]
                    nop.sync_info = mybir.SyncInfo(on_wait=[w], on_update=[])
                    new_list.append(nop)
                ins.sync_info = mybir.SyncInfo(
                    on_wait=[waits[-1]], on_update=list(si.on_update or []))
                changed = True
            new_list.append(ins)
        if changed:
            b.instructions = new_list
        for sub in getattr(b, "blocks", []) or []:
            process(sub)

    for b in f.blocks:
        process(b)


def _build_nc():
    rows = P * R
    nc = bass.Bass()
    v1 = nc.declare_dram_parameter("volume1", [rows, 2], F32, isOutput=False)
    v2 = nc.declare_dram_parameter("volume2", [rows, 2], F32, isOutput=False)
    v3 = nc.declare_dram_parameter("volume3", [rows, 2], F32, isOutput=False)
    xy = nc.declare_dram_parameter("xy_rel_id", [rows, 2], I32, isOutput=False)
    yz = nc.declare_dram_parameter("yz_rel_id", [rows, 2], I32, isOutput=False)
    xz = nc.declare_dram_parameter("xz_rel_id", [rows, 2], I32, isOutput=False)
    fl = nc.declare_dram_parameter("flag", [rows], I32, isOutput=False)
    n_chunks = R // min(NEG_CHUNK, R)
    chunk = R // n_chunks
    # first tile split into smaller prologue slices to prime the
    # ACT->POOL->DVE pipeline sooner
    slices = PROLOGUE_SLICES + [(o, N_TILE) for o in range(PROLOGUE_ROWS, R, N_TILE)]
    pos_out = nc.declare_dram_parameter("pos", [P, len(slices) * N_SETS], F32, isOutput=True)
    cnt_out = nc.declare_dram_parameter("cnt", [P, n_chunks * N_NEG], F32, isOutput=True)

    v1r = v1.rearrange("(p n) m -> p n m", p=P)
    v2r = v2.rearrange("(p n) m -> p n m", p=P)
    v3r = v3.rearrange("(p n) m -> p n m", p=P)
    xyr = xy.rearrange("(p n) m -> p n m", p=P)
    yzr = yz.rearrange("(p n) m -> p n m", p=P)
    xzr = xz.rearrange("(p n) m -> p n m", p=P)
    flr = fl.rearrange("(p n) -> p n", p=P)
    N = N_TILE

    with tile.TileContext(nc) as tc:
        with tc.tile_pool(name="io", bufs=3) as io, \
             tc.tile_pool(name="scr", bufs=2) as scr, \
             tc.tile_pool(name="accs", bufs=1) as accs:
            pos_acc = accs.tile([P, len(slices) * N_SETS], F32)
            cnt_acc = accs.tile([P, n_chunks * N_NEG], F32)
            K_full = accs.tile([P, R], F16)

            for j, (off, N) in enumerate(slices):
                sl = slice(off, off + N)
                v1t = io.tile([P, N, 2], F32, tag="v1t")
                v2t = io.tile([P, N, 2], F32, tag="v2t")
                v3t = io.tile([P, N, 2], F32, tag="v3t")
                xyt = io.tile([P, N, 2], I32, tag="xyt")
                yzt = io.tile([P, N, 2], I32, tag="yzt")
                xzt = io.tile([P, N, 2], I32, tag="xzt")
                flt = io.tile([P, N], I32, tag="flt")
                for nm, dst, src_ap in (("v1t", v1t, v1r[:, sl, :]),
                                        ("v2t", v2t, v2r[:, sl, :]),
                                        ("v3t", v3t, v3r[:, sl, :]),
                                        ("xyt", xyt, xyr[:, sl, :]),
                                        ("yzt", yzt, yzr[:, sl, :]),
                                        ("xzt", xzt, xzr[:, sl, :]),
                                        ("flt", flt, flr[:, sl])):
                    eng = nc.scalar if nm in ACT_LOADS else nc.sync
                    eng.dma_start(dst[:], src_ap)

                # K = wx + wy + wz + (292*flag + 109.5), w = (r0-.5)(4s*r1-3s)
                us, vs = [], []
                for nm, rel, s in (("x", xyt, 1.0), ("y", yzt, 8.0), ("z", xzt, 64.0)):
                    u = scr.tile([P, N], F16, tag=f"u{nm}")
                    v = scr.tile([P, N], F16, tag=f"v{nm}")
                    nc.scalar.activation(u[:], rel[:, :, 0], AF.Copy, bias=-0.5, scale=1.0)
                    nc.scalar.activation(v[:], rel[:, :, 1], AF.Copy, bias=-3.0 * s, scale=4.0 * s)
                    us.append(u); vs.append(v)
                ff = scr.tile([P, N], F16, tag="ff")
                nc.scalar.activation(ff[:], flt[:], AF.Copy, bias=109.5, scale=292.0)
                for u, v in zip(us, vs):
                    nc.gpsimd.tensor_tensor(u[:], u[:], v[:], ALU.mult)
                nc.gpsimd.tensor_tensor(us[0][:], us[0][:], us[1][:], ALU.add)
                nc.gpsimd.tensor_tensor(us[2][:], us[2][:], ff[:], ALU.add)
                Ksl = K_full[:, sl]
                nc.gpsimd.tensor_tensor(Ksl, us[0][:], us[2][:], ALU.add)

                # positive branch: masked sums per key-group
                adds_left = DVE_ADDS
                for s, (keys, (a, b, c)) in enumerate(POS_SETS):
                    M = scr.tile([P, N], F16, tag=f"M{s}")
                    nc.vector.tensor_scalar(M[:], Ksl, float(keys[0]), None, ALU.is_equal)
                    for ki, k in enumerate(keys[1:]):
                        CMP = scr.tile([P, N], F16, tag=f"CMP{ki % 4}")
                        nc.vector.tensor_scalar(CMP[:], Ksl, float(k), None, ALU.is_equal)
                        if adds_left > 0:
                            nc.vector.tensor_tensor(M[:], M[:], CMP[:], ALU.add)
                            adds_left -= 1
                        else:
                            nc.gpsimd.tensor_tensor(M[:], M[:], CMP[:], ALU.add)
                    TT = scr.tile([P, N], F32, tag=f"T{s}")
                    nc.gpsimd.tensor_tensor(TT[:], v1t[:, :, a], v2t[:, :, b], ALU.add)
                    nc.gpsimd.tensor_tensor(TT[:], TT[:], v3t[:, :, c], ALU.subtract)
                    D = scr.tile([P, N], F32, tag="D")
                    # scalar_tensor_tensor is DVE-only in this codegen
                    nc.vector.scalar_tensor_tensor(
                        D[:], TT[:], 1.0, M[:], ALU.mult, ALU.mult,
                        accum_out=pos_acc[:, j * N_SETS + s:j * N_SETS + s + 1])

                # negative branch: per-recipe match counts over a coarser
                # chunk of K (compare + fused per-partition sum; op1 is the
                # reduction operator). Coarser tiles amortize DVE per-op cost.
                if (off + N) % chunk == 0:
                    c2 = (off + N) // chunk - 1
                    Kch = K_full[:, c2 * chunk:(c2 + 1) * chunk]
                    NS = scr.tile([P, chunk], F16, tag="NS")
                    for r, k in enumerate(NEG_KEYS):
                        nc.vector.tensor_scalar(
                            NS[:], Kch, float(k), None, ALU.is_equal, ALU.add,
                            accum_out=cnt_acc[:, c2 * N_NEG + r:c2 * N_NEG + r + 1])

            nc.sync.dma_start(pos_out[:], pos_acc[:])
            nc.scalar.dma_start(cnt_out[:], cnt_acc[:])

    _split_multi_waits(nc)
    return nc


_NC_CACHE = None


def _get_nc():
    global _NC_CACHE
    if _NC_CACHE is None:
        _NC_CACHE = _build_nc()
    return _NC_CACHE


# ------------------------- host-side helpers ------------------------------
def _codes_np(rel, flag):
    r0, r1 = rel[:, 0], rel[:, 1]
    cls = np.where((r0 == 1) & (r1 == 0), 0,
          np.where((r0 == 0) & (r1 == 1), 1,
          np.where((r0 == 1) & (r1 == 1), 2, 3)))
    return cls + 4 * flag


def _log1mexp_np(x):
    x = np.asarray(x, dtype=np.float32)
    return np.where(x > np.float32(LOG_HALF),
                    np.log(-np.expm1(x)), np.log1p(-np.exp(x))).astype(np.float32)


def _neg_term_host(volume1, volume2, volume3, cx, cy, cz, xy, yz, xz):
    """Exact reference semantics for one negative recipe (used only when the
    device-computed count for that recipe is non-zero)."""
    m = (cx == xy) & (cy == yz) & (cz == xz)
    cs = np.cumsum(m.astype(np.int32))
    count = int(cs[-1])
    if count <= 0:
        return np.float32(0.0)
    f1, f2, f3 = xy // 4, yz // 4, xz // 4
    i1 = int(np.argmax(cs == f1 + 1))
    i2 = int(np.argmax(cs == f2 + 1))
    i3 = int(np.argmax(cs == f3 + 1))
    term = (volume1[i1].astype(np.float32)
            + volume2[i2].astype(np.float32)
            - _log1mexp_np(volume3[i3])).sum(dtype=np.float32)
    return np.float32(term)


def kernel(volume1, volume2, volume3, xy_rel_id, yz_rel_id, xz_rel_id, flag):
    v1 = np.ascontiguousarray(np.asarray(volume1, dtype=np.float32))
    v2 = np.ascontiguousarray(np.asarray(volume2, dtype=np.float32))
    v3 = np.ascontiguousarray(np.asarray(volume3, dtype=np.float32))
    xy = np.ascontiguousarray(np.asarray(xy_rel_id).astype(np.int32, copy=False))
    yz = np.ascontiguousarray(np.asarray(yz_rel_id).astype(np.int32, copy=False))
    xz = np.ascontiguousarray(np.asarray(xz_rel_id).astype(np.int32, copy=False))
    fl = np.ascontiguousarray(np.asarray(flag).astype(np.int32, copy=False))
    assert v1.shape == (B, 2) and fl.shape == (B,)

    nc = _get_nc()
    S = ROWS_PER_CORE
    in_maps = [{
        "volume1": v1[c * S:(c + 1) * S],
        "volume2": v2[c * S:(c + 1) * S],
        "volume3": v3[c * S:(c + 1) * S],
        "xy_rel_id": xy[c * S:(c + 1) * S],
        "yz_rel_id": yz[c * S:(c + 1) * S],
        "xz_rel_id": xz[c * S:(c + 1) * S],
        "flag": fl[c * S:(c + 1) * S],
    } for c in range(N_CORES)]

    res = run_bass_kernel_spmd(nc, in_maps, core_ids=list(range(N_CORES)))

    pos_total = np.float32(0.0)
    counts = np.zeros(N_NEG, dtype=np.float64)
    n_chunks = R // min(NEG_CHUNK, R)
    for c in range(N_CORES):
        pos = res.results[c]["pos"]          # [P, T * N_SETS]
        cnt = res.results[c]["cnt"]          # [P, n_chunks * N_NEG]
        pos_total = np.float32(pos_total + pos.sum(dtype=np.float64))
        counts += cnt.reshape(P, n_chunks, N_NEG).sum(axis=(0, 1))

    loss = np.float32(0.0) - pos_total

    if np.any(counts > 0):
        # some negative-recipe mask is non-empty: resolve those recipes on
        # the host with exact reference semantics
        cx = _codes_np(xy, fl)
        cy = _codes_np(yz, fl)
        cz = _codes_np(xz, fl)
        for r, (rxy, ryz, rxz) in enumerate(NEG_LOSS_RECIPE):
            if counts[r] > 0:
                loss = np.float32(loss - _neg_term_host(v1, v2, v3, cx, cy, cz,
                                                        rxy, ryz, rxz))

    return np.float32(loss)
